# revision 1
# baseline (speedup 1.0000x reference)
"""HGT layer (heterogeneous graph transformer) on 8 Trainium2 NeuronCores.

Strategy (dst-partitioned, per sharding hint):
  - Destination nodes are partitioned contiguously across the 8 cores
    (papers 12500/core, authors 6250/core). All edges of a dst live on its
    owner core, so edge softmax + scatter-sum are fully local.
  - Host-side prep ("halo gather"): per core, edges are grouped by dst tile
    (128 dsts), padded to uniform per-tile block budgets (max over cores so
    one SPMD program serves all cores), and the source-node features are
    pre-gathered into transposed [in=128, edge=128] blocks for streaming.
  - Device: per 128-edge block
      rec  = hsrcT.T @ Wkv            (k~ and v~ per edge, PSUM f32)
      A    = onehot(dst_lane)         (iota == dst compare, bf16)
      At   = A.T                      (PE transpose)
      qx   = At.T @ Q                 (per-edge q via one-hot matmul)
      score= rowsum4(rec_k * qx);  e = exp(score)
      msg  = [rec_v * e | e]
      agg += A.T @ msg                (segment-sum + softmax denom, PSUM)
    Per 128-dst tile: normalize by 1/z, combine relations, transpose,
    out = T.T @ WaT + (1-alpha) * h, DMA out.
  Weight folding (host): rel_att/rel_msg folded into Wk/Wv per relation;
  rel_pri/sqrt(dk) folded into the attention weights; alpha=sigmoid(skip)
  and the 0.5 cross-relation mean folded into Wa.
"""

import math
import os

import numpy as np
import ml_dtypes

BF16 = ml_dtypes.bfloat16

NPAP, NAUT = 100000, 50000
D, H, DK = 128, 4, 32
NCORES = 8
PPC, APC = NPAP // NCORES, NAUT // NCORES  # 12500, 6250
PT = (PPC + 127) // 128  # 98 paper tiles / core
AT = (APC + 127) // 128  # 49 author tiles / core
GH = 8   # hsrcT blocks per DMA group
GD = 64  # dst blocks per DMA group

LAST_RESULT = {}


def _prep_relation(src, dst, h_src_ext, n_per_core, ntiles):
    """Partition edges by dst owner core, group by dst tile, pad to uniform
    budgets. Returns (nblk[t] budgets, per-core hsrcT [NB,128,128] bf16,
    per-core dstT [128, NB] f32)."""
    core = dst // n_per_core
    dloc = dst - core * n_per_core
    tl = dloc >> 7
    lane = (dloc & 127).astype(np.float32)

    cnt = np.bincount(core * ntiles + tl, minlength=NCORES * ntiles).reshape(
        NCORES, ntiles
    )
    nblk = (cnt.max(axis=0) + 127) // 128  # blocks per tile (uniform)
    NB = int(nblk.sum())
    tile_slot0 = np.concatenate([[0], np.cumsum(nblk)]) * 128

    hsT_cores, dstT_cores, at_cores = [], [], []
    zero_row = h_src_ext.shape[0] - 1  # h_src_ext has appended zero row
    for c in range(NCORES):
        sel = np.nonzero(core == c)[0]
        tl_c = tl[sel]
        order = np.argsort(tl_c, kind="stable")
        sel_o = sel[order]
        tl_s = tl_c[order]
        start_of = np.searchsorted(tl_s, np.arange(ntiles))
        within = np.arange(len(sel_o)) - start_of[tl_s]
        slot = tile_slot0[tl_s] + within

        src_slots = np.full(NB * 128, zero_row, np.int64)
        src_slots[slot] = src[sel_o]
        lane_slots = np.full(NB * 128, 255.0, np.float32)
        lane_slots[slot] = lane[sel_o]

        mat = h_src_ext[src_slots]  # [NB*128, 128] f32
        hsT = np.ascontiguousarray(
            mat.reshape(NB, 128, 128).transpose(0, 2, 1)
        ).astype(BF16)
        dstT = np.ascontiguousarray(lane_slots.reshape(NB, 128).T)
        # A_T[b, d, e] = 1 if dst_lane(b, e) == d   (pads hit no row)
        at = (
            np.arange(128, dtype=np.float32)[None, :, None]
            == lane_slots.reshape(NB, 1, 128)
        ).astype(BF16)
        hsT_cores.append(hsT)
        dstT_cores.append(dstT)
        at_cores.append(np.ascontiguousarray(at))
    return nblk, NB, hsT_cores, dstT_cores, at_cores


def _prep_dst_type(h, n_per_core, ntiles):
    """Per-core dst-node features: transposed bf16 (for Q / matmul) and
    row-major f32 (for the skip blend)."""
    hdT, hrow = [], []
    for c in range(NCORES):
        rows = h[c * n_per_core : (c + 1) * n_per_core]
        pad = np.zeros((ntiles * 128, D), np.float32)
        pad[: rows.shape[0]] = rows
        t = pad.reshape(ntiles, 128, D)
        hdT.append(np.ascontiguousarray(t.transpose(0, 2, 1)).astype(BF16))
        hrow.append(np.ascontiguousarray(t))
    return hdT, hrow


def _fold_weights(Wk, Wv, Wq, Wa, rel_att, rel_msg, rel_pri, skip):
    """Fold per-relation transforms into the projection weights."""
    sqrt_dk = math.sqrt(DK)
    # relation -> (src node type)
    rel_ts = [0, 1, 0]  # cites: paper, writes: author, rev: paper
    wkv = []
    for e in range(3):
        ts = rel_ts[e]
        ratt = rel_att[e] * (rel_pri[e][:, None, None] / sqrt_dk)
        watt = np.einsum(
            "hiI,hij->Ihj", Wk[ts].reshape(H, DK, D), ratt
        ).reshape(D, D)
        wmsg = np.einsum(
            "hiI,hij->Ihj", Wv[ts].reshape(H, DK, D), rel_msg[e]
        ).reshape(D, D)
        wkv.append(np.ascontiguousarray(np.concatenate([watt, wmsg], 1)).astype(BF16))
    wq = [np.ascontiguousarray(Wq[t].T).astype(BF16) for t in range(2)]
    alpha = 1.0 / (1.0 + np.exp(-skip.astype(np.float64)))
    waT = [
        np.ascontiguousarray(Wa[0].T * alpha[0] * 0.5).astype(BF16),
        np.ascontiguousarray(Wa[1].T * alpha[1]).astype(BF16),
    ]
    return wkv, wq, waT, alpha


def kernel(**inputs):
    from concourse import bacc, bass, mybir, tile
    from concourse.bass_utils import run_bass_kernel_spmd

    inp = {k: np.asarray(v) for k, v in inputs.items()}
    h_paper = inp["h_paper"].astype(np.float32)
    h_author = inp["h_author"].astype(np.float32)
    for bname in ("bk", "bq", "bv", "ba"):
        assert not np.any(inp[bname]), f"nonzero bias {bname} unsupported"

    wkv, wq, waT, alpha = _fold_weights(
        inp["Wk"].astype(np.float32), inp["Wv"].astype(np.float32),
        inp["Wq"].astype(np.float32), inp["Wa"].astype(np.float32),
        inp["rel_att"].astype(np.float32), inp["rel_msg"].astype(np.float32),
        inp["rel_pri"].astype(np.float32), inp["skip"].astype(np.float32),
    )

    hp_ext = np.concatenate([h_paper, np.zeros((1, D), np.float32)], 0)
    ha_ext = np.concatenate([h_author, np.zeros((1, D), np.float32)], 0)

    # relations: name, src, dst, h_src_ext, dst n/core, dst ntiles
    nblk_c, NBC, hsT_c, dstT_c, at_c = _prep_relation(
        inp["cites_src"].astype(np.int64), inp["cites_dst"].astype(np.int64),
        hp_ext, PPC, PT)
    nblk_w, NBW, hsT_w, dstT_w, at_w = _prep_relation(
        inp["writes_src"].astype(np.int64), inp["writes_dst"].astype(np.int64),
        ha_ext, PPC, PT)
    nblk_r, NBR, hsT_r, dstT_r, at_r = _prep_relation(
        inp["rev_src"].astype(np.int64), inp["rev_dst"].astype(np.int64),
        hp_ext, APC, AT)

    hdT_p, hrow_p = _prep_dst_type(h_paper, PPC, PT)
    hdT_a, hrow_a = _prep_dst_type(h_author, APC, AT)

    # ---------------- build the SPMD Bass program ----------------
    nc = bacc.Bacc("TRN2", target_bir_lowering=False, debug=False,
                   num_devices=NCORES)
    dt = mybir.dt

    d_hsT = {
        "cites": nc.dram_tensor("hsT_cites", [max(NBC, 1), 128, 128], dt.bfloat16,
                                kind="ExternalInput"),
        "writes": nc.dram_tensor("hsT_writes", [max(NBW, 1), 128, 128], dt.bfloat16,
                                 kind="ExternalInput"),
        "rev": nc.dram_tensor("hsT_rev", [max(NBR, 1), 128, 128], dt.bfloat16,
                              kind="ExternalInput"),
    }
    d_dstT = {
        "cites": nc.dram_tensor("dstT_cites", [128, max(NBC, 1)], dt.float32,
                                kind="ExternalInput"),
        "writes": nc.dram_tensor("dstT_writes", [128, max(NBW, 1)], dt.float32,
                                 kind="ExternalInput"),
        "rev": nc.dram_tensor("dstT_rev", [128, max(NBR, 1)], dt.float32,
                              kind="ExternalInput"),
    }
    d_at = {
        "cites": nc.dram_tensor("at_cites", [max(NBC, 1), 128, 128], dt.bfloat16,
                                kind="ExternalInput"),
        "writes": nc.dram_tensor("at_writes", [max(NBW, 1), 128, 128],
                                 dt.bfloat16, kind="ExternalInput"),
        "rev": nc.dram_tensor("at_rev", [max(NBR, 1), 128, 128], dt.bfloat16,
                              kind="ExternalInput"),
    }
    d_hdT = {
        0: nc.dram_tensor("hdT_paper", [PT, 128, 128], dt.bfloat16,
                          kind="ExternalInput"),
        1: nc.dram_tensor("hdT_author", [AT, 128, 128], dt.bfloat16,
                          kind="ExternalInput"),
    }
    d_hrow = {
        0: nc.dram_tensor("hrow_paper", [PT, 128, 128], dt.float32,
                          kind="ExternalInput"),
        1: nc.dram_tensor("hrow_author", [AT, 128, 128], dt.float32,
                          kind="ExternalInput"),
    }
    NOUT = (PT + AT) * 128
    d_out = nc.dram_tensor("out", [NOUT, 128], dt.float32, kind="ExternalOutput")

    debug_dump = bool(int(os.environ.get("HGT_DEBUG_DUMP", "0")))
    d_dbg = {}
    if debug_dump:
        for nm, w in [("rec", 256), ("A", 128), ("qx", 128), ("prod", 128),
                      ("scores", 4), ("esc", 4), ("msg", 132), ("Q", 128),
                      ("agg", 132), ("hs", 128), ("At", 128), ("dcol", 1)]:
            d_dbg[nm] = nc.dram_tensor(f"dbg_{nm}", [128, w], dt.float32,
                                       kind="ExternalOutput")

    d_wkv = [nc.inline_tensor(wkv[e], name=f"wkv{e}") for e in range(3)]
    d_wq = [nc.inline_tensor(wq[t], name=f"wq{t}") for t in range(2)]
    d_waT = [nc.inline_tensor(waT[t], name=f"waT{t}") for t in range(2)]
    iota_np = np.tile(np.arange(128, dtype=np.float32), (128, 1))
    d_iota = nc.inline_tensor(iota_np, name="iotac")
    d_ident = nc.inline_tensor(np.eye(128, dtype=np.float32).astype(BF16),
                               name="identc")

    # rel name -> (dram hsT, dram dstT, dram A_T, budgets, wkv idx)
    rel_info = {
        "cites": (d_hsT["cites"], d_dstT["cites"], d_at["cites"], nblk_c, 0),
        "writes": (d_hsT["writes"], d_dstT["writes"], d_at["writes"], nblk_w, 1),
        "rev": (d_hsT["rev"], d_dstT["rev"], d_at["rev"], nblk_r, 2),
    }

    with tile.TileContext(nc) as tc:
        with (
            tc.tile_pool(name="const", bufs=1) as cpool,
            tc.tile_pool(name="hs", bufs=4) as hs_pool,
            tc.tile_pool(name="dstg", bufs=2) as dst_pool,
            tc.tile_pool(name="work", bufs=3) as wpool,
            tc.tile_pool(name="tilew", bufs=3) as tpool,
            tc.tile_pool(name="rec_ps", bufs=2, space="PSUM") as rec_ps,
            tc.tile_pool(name="qx_ps", bufs=2, space="PSUM") as qx_ps,
            tc.tile_pool(name="agg_ps", bufs=2, space="PSUM") as agg_ps,
            tc.tile_pool(name="o_ps", bufs=1, space="PSUM") as o_ps,
        ):
            # constants to SBUF
            s_wkv = []
            for e in range(3):
                w = cpool.tile([128, 256], dt.bfloat16, name=f"s_wkv{e}")
                nc.sync.dma_start(out=w[:], in_=d_wkv[e][:])
                s_wkv.append(w)
            s_wq, s_waT = [], []
            for t in range(2):
                a = cpool.tile([128, 128], dt.bfloat16, name=f"s_wq{t}")
                nc.sync.dma_start(out=a[:], in_=d_wq[t][:])
                s_wq.append(a)
                b = cpool.tile([128, 128], dt.bfloat16, name=f"s_waT{t}")
                nc.sync.dma_start(out=b[:], in_=d_waT[t][:])
                s_waT.append(b)
            s_iota = cpool.tile([128, 128], dt.float32, name="s_iota")
            nc.sync.dma_start(out=s_iota[:], in_=d_iota[:])
            s_ident = cpool.tile([128, 128], dt.bfloat16, name="s_ident")
            nc.sync.dma_start(out=s_ident[:], in_=d_ident[:])

            # streaming group state per relation
            gstate = {r: {"g": 0, "hs": None, "at": None, "dst": None}
                      for r in rel_info}

            def get_block(rname):
                st = gstate[rname]
                d_hs, d_dst, d_att, _, _ = rel_info[rname]
                g = st["g"]
                hi, ho = divmod(g, GH)
                if ho == 0:
                    nb = d_hs.shape[0]
                    n = min(GH, nb - hi * GH)
                    hsg = hs_pool.tile([128, GH, 128], dt.bfloat16, name="hsg",
                                       tag="hsg")
                    nc.sync.dma_start(
                        out=hsg[:, :n, :],
                        in_=d_hs[hi * GH : hi * GH + n, :, :].rearrange(
                            "b p c -> p b c"),
                    )
                    st["hs"] = hsg
                    atg = hs_pool.tile([128, GH, 128], dt.bfloat16, name="atg",
                                       tag="atg")
                    nc.sync.dma_start(
                        out=atg[:, :n, :],
                        in_=d_att[hi * GH : hi * GH + n, :, :].rearrange(
                            "b p c -> p b c"),
                    )
                    st["at"] = atg
                di, do = divmod(g, GD)
                if do == 0:
                    nb = d_dst.shape[1]
                    n = min(GD, nb - di * GD)
                    dg = dst_pool.tile([128, GD], dt.float32, name="dg", tag="dg")
                    nc.sync.dma_start(
                        out=dg[:, :n], in_=d_dst[:, di * GD : di * GD + n]
                    )
                    st["dst"] = dg
                st["g"] = g + 1
                return (st["hs"][:, ho, :], st["at"][:, ho, :],
                        st["dst"][:, do : do + 1])

            def dump(nm, ap):
                w = d_dbg[nm].shape[1]
                tmp = wpool.tile([128, w], dt.float32, name=f"dmp_{nm}",
                                 tag=f"dmp_{nm}")
                nc.vector.tensor_copy(out=tmp[:], in_=ap)
                nc.sync.dma_start(out=d_dbg[nm][:, :], in_=tmp[:])

            def do_tile(ttype, ti, rels):
                # Q for this dst tile
                q_ps = o_ps.tile([128, 128], dt.float32, name="q_ps", tag="ops")
                hdt = tpool.tile([128, 128], dt.bfloat16, name="hdt", tag="hdt")
                nc.sync.dma_start(out=hdt[:], in_=d_hdT[ttype][ti, :, :])
                nc.tensor.matmul(q_ps[:], lhsT=hdt[:], rhs=s_wq[ttype][:],
                                 start=True, stop=True)
                Q = tpool.tile([128, 128], dt.bfloat16, name="Q", tag="Q")
                nc.scalar.copy(out=Q[:], in_=q_ps[:])
                if debug_dump and ttype == 0 and ti == 0:
                    dump("Q", Q[:])

                aggs = []
                for rname in rels:
                    _, _, _, nblk, widx = rel_info[rname]
                    nb = int(nblk[ti])
                    if nb == 0:
                        aggs.append(None)
                        continue
                    agg = agg_ps.tile([128, 132], dt.float32, name="agg",
                                      tag="agg")
                    for b in range(nb):
                        hs, At, dcol = get_block(rname)
                        rec = rec_ps.tile([128, 256], dt.float32, name="rec",
                                          tag="rec")
                        nc.tensor.matmul(rec[:], lhsT=hs, rhs=s_wkv[widx][:],
                                         start=True, stop=True)
                        A = wpool.tile([128, 128], dt.bfloat16, name="A", tag="A")
                        nc.vector.tensor_scalar(
                            out=A[:], in0=s_iota[:], scalar1=dcol, scalar2=None,
                            op0=mybir.AluOpType.is_equal)
                        qx = qx_ps.tile([128, 128], dt.float32, name="qx",
                                        tag="qx")
                        nc.tensor.matmul(qx[:], lhsT=At, rhs=Q[:],
                                         start=True, stop=True)
                        qxs = wpool.tile([128, 128], dt.float32, name="qxs",
                                         tag="qxs")
                        nc.scalar.copy(out=qxs[:], in_=qx[:])
                        prod = wpool.tile([128, 128], dt.float32, name="prod",
                                          tag="prod")
                        nc.vector.tensor_tensor(
                            out=prod[:], in0=rec[:, 0:128], in1=qxs[:],
                            op=mybir.AluOpType.mult)
                        scores = wpool.tile([128, 4], dt.float32, name="scores",
                                            tag="scores")
                        nc.vector.tensor_reduce(
                            out=scores[:],
                            in_=prod[:].rearrange("p (h i) -> p h i", h=4),
                            axis=mybir.AxisListType.X, op=mybir.AluOpType.add)
                        msg = wpool.tile([128, 132], dt.bfloat16, name="msg",
                                         tag="msg")
                        esc = wpool.tile([128, 4], dt.float32, name="esc",
                                         tag="esc")
                        nc.scalar.activation(
                            out=esc[:], in_=scores[:],
                            func=mybir.ActivationFunctionType.Exp)
                        nc.scalar.copy(out=msg[:, 128:132], in_=esc[:])
                        for h in range(4):
                            nc.vector.tensor_scalar(
                                out=msg[:, 32 * h : 32 * h + 32],
                                in0=rec[:, 128 + 32 * h : 160 + 32 * h],
                                scalar1=esc[:, h : h + 1], scalar2=None,
                                op0=mybir.AluOpType.mult)
                        nc.tensor.matmul(agg[:], lhsT=A[:], rhs=msg[:],
                                         start=(b == 0), stop=(b == nb - 1))
                        if (debug_dump and ttype == 0 and ti == 0
                                and rname == "cites"):
                            if b == 0:
                                dump("hs", hs)
                                dump("At", At)
                                dump("dcol", dcol)
                                dump("rec", rec[:])
                                dump("A", A[:])
                                dump("qx", qx[:])
                                dump("prod", prod[:])
                                dump("scores", scores[:])
                                dump("esc", esc[:])
                                dump("msg", msg[:])
                            if b == nb - 1:
                                dump("agg", agg[:])
                    aggs.append(agg)

                # finalize tile
                Ts = []
                for agg in aggs:
                    if agg is None:
                        continue
                    zb = wpool.tile([128, 4], dt.float32, name="zb", tag="zb")
                    nc.vector.tensor_scalar(
                        out=zb[:], in0=agg[:, 128:132], scalar1=1e-30,
                        scalar2=None, op0=mybir.AluOpType.add)
                    rz = wpool.tile([128, 4], dt.float32, name="rz", tag="zb")
                    nc.vector.reciprocal(out=rz[:], in_=zb[:])
                    T = tpool.tile([128, 128], dt.bfloat16, name="T", tag="T")
                    for h in range(4):
                        nc.vector.tensor_scalar(
                            out=T[:, 32 * h : 32 * h + 32],
                            in0=agg[:, 32 * h : 32 * h + 32],
                            scalar1=rz[:, h : h + 1], scalar2=None,
                            op0=mybir.AluOpType.mult)
                    Ts.append(T)

                orow = ti * 128 if ttype == 0 else (PT + ti) * 128
                out_s = tpool.tile([128, 128], dt.float32, name="out_s",
                                   tag="out_s")
                hrow = tpool.tile([128, 128], dt.float32, name="hrow",
                                  tag="hrow")
                nc.sync.dma_start(out=hrow[:], in_=d_hrow[ttype][ti, :, :])
                if Ts:
                    Tc = Ts[0]
                    if len(Ts) == 2:
                        Tsum = tpool.tile([128, 128], dt.bfloat16, name="Tsum",
                                          tag="Tsum")
                        nc.vector.tensor_tensor(out=Tsum[:], in0=Ts[0][:],
                                                in1=Ts[1][:],
                                                op=mybir.AluOpType.add)
                        Tc = Tsum
                    tt_ps = qx_ps.tile([128, 128], dt.bfloat16, name="tt_ps",
                                       tag="qx")
                    nc.tensor.transpose(tt_ps[:], Tc[:], s_ident[:])
                    Tt = tpool.tile([128, 128], dt.bfloat16, name="Tt", tag="Tt")
                    nc.scalar.copy(out=Tt[:], in_=tt_ps[:])
                    out_ps = o_ps.tile([128, 128], dt.float32, name="out_ps",
                                       tag="ops")
                    nc.tensor.matmul(out_ps[:], lhsT=Tt[:], rhs=s_waT[ttype][:],
                                     start=True, stop=True)
                    nc.vector.scalar_tensor_tensor(
                        out=out_s[:], in0=hrow[:],
                        scalar=float(1.0 - alpha[ttype]), in1=out_ps[:],
                        op0=mybir.AluOpType.mult, op1=mybir.AluOpType.add)
                else:
                    nc.vector.tensor_scalar(
                        out=out_s[:], in0=hrow[:],
                        scalar1=float(1.0 - alpha[ttype]), scalar2=None,
                        op0=mybir.AluOpType.mult)
                nc.sync.dma_start(out=d_out[orow : orow + 128, :], in_=out_s[:])

            for ti in range(PT):
                do_tile(0, ti, ["cites", "writes"])
            for ti in range(AT):
                do_tile(1, ti, ["rev"])

    nc.compile()

    if os.environ.get("HGT_BUILD_ONLY"):
        return np.zeros((NPAP + NAUT, D), np.float32)

    in_maps = []
    for c in range(NCORES):
        in_maps.append({
            "hsT_cites": hsT_c[c], "hsT_writes": hsT_w[c], "hsT_rev": hsT_r[c],
            "dstT_cites": dstT_c[c], "dstT_writes": dstT_w[c],
            "dstT_rev": dstT_r[c],
            "at_cites": at_c[c], "at_writes": at_w[c], "at_rev": at_r[c],
            "hdT_paper": hdT_p[c], "hdT_author": hdT_a[c],
            "hrow_paper": hrow_p[c], "hrow_author": hrow_a[c],
        })

    trace = bool(int(os.environ.get("HGT_TRACE", "0")))
    res = run_bass_kernel_spmd(nc, in_maps, list(range(NCORES)), trace=trace)
    LAST_RESULT["exec_time_ns"] = res.exec_time_ns
    LAST_RESULT["res"] = res
    LAST_RESULT["nc"] = nc
    LAST_RESULT["in_maps"] = in_maps

    out = np.empty((NPAP + NAUT, D), np.float32)
    for c in range(NCORES):
        o = np.asarray(res.results[c]["out"], np.float32)
        out[c * PPC : (c + 1) * PPC] = o[:PPC]
        out[NPAP + c * APC : NPAP + (c + 1) * APC] = o[PT * 128 : PT * 128 + APC]
    return out



# revision 10
# speedup vs baseline: 1.5504x; 1.5504x over previous
"""HGT layer (heterogeneous graph transformer) on 8 Trainium2 NeuronCores.

v2: engine-balanced redesign (v1 was DVE-bound at 3.1ms: ~8.5 vector ops
per 128-edge block, each paying ~150cyc fixed overhead).

Strategy (dst-partitioned, per sharding hint):
  - Dst nodes partitioned contiguously across 8 cores. Host groups edges
    by dst tile (128 dsts), pads to uniform per-tile block budgets, and
    pre-gathers per-edge data into three flat streams (cols = edge slots,
    in flat schedule order):
      hsT [128=feat, NBF*128]   source features, transposed, bf16
      at  [128=dlane, NBF*128]  one-hot A^T (dst lane per edge), bf16
      Aa  [128=elane, NBF*128]  one-hot A (per block: A[e, b*128+d]), bf16
  - Device, per 4-block group (512 edges), scores in TRANSPOSED layout
    (features on partitions, edges on free axis) so DVE fixed costs
    amortize 4x:
      kT   = watt.T @ hsT4          (PE, wkv stationary)
      qxT  = Q.T @ at4              (PE, per-tile Q stationary)
      prodT= kT * qxT               (DVE, one op per 4 blocks, fp16 out)
      scores[4s+h, e] += Hmask64_s.T @ prodT   (PE, per-head col sums)
    Per 16 groups (superblock): one ACT exp -> escT fp16.
  - Per block (edge-major message path):
      esc_full = escT_slice.T @ HselI_s   (PE: [e,132] = esc expanded to
                 per-head cols 0:128 + raw esc at cols 128:132)
      v        = hsT_b.T @ wmsg           (PE, into bank cols 132:260;
                 cols 260:264 pre-set to 1.0)
      msg      = v132 * esc_full          (DVE, one op: [e,0:128]=v*esc,
                 [e,128:132]=esc)
      aggT[f,d] += msg[:,0:128].T @ A_b   (PE, transposed scatter-sum)
      zT[h,d]  += msg[:,128:132].T @ A_b  (PE, softmax denominators)
  - Per dst tile: rz = exp(-ln(z+eps)) on ACT (no table switches:
    ln/exp/copy share one ACT table set), rz_expT = Hsel4.T @ rzT (PE
    partition-broadcast), T = aggT*rz_expT (DVE), out += T.T@WaT (PE,
    accumulated over relations), blend with skip (DVE stt), DMA out.
  Weight folds as v1: rel_att/rel_msg into Wk/Wv; pri/sqrt(dk) into
  attention weights; sigmoid(skip) and 0.5 cross-relation mean into Wa.
"""

import math
import os

import numpy as np
import ml_dtypes

BF16 = ml_dtypes.bfloat16
FP16 = np.float16

NPAP, NAUT = 100000, 50000
D, H, DK = 128, 4, 32
NCORES = 8
PPC, APC = NPAP // NCORES, NAUT // NCORES  # 12500, 6250
PT = (PPC + 127) // 128  # 98 paper tiles / core
AT = (APC + 127) // 128  # 49 author tiles / core

G = 4           # blocks per score group
NSLOT = 16      # groups per superblock (scores psum tile rows = 4*NSLOT)
CHUNK = 32      # blocks per DMA chunk

LAST_RESULT = {}


def _edge_slots(src, dst, n_per_core, ntiles, zero_row):
    """Per-core edge slot assignment grouped by dst tile with uniform
    per-tile block budgets (max over cores). Returns nblk[t] and per-core
    (src_slots, lane_slots) flat arrays of length NB*128."""
    core = dst // n_per_core
    dloc = dst - core * n_per_core
    tl = dloc >> 7
    lane = (dloc & 127).astype(np.int32)

    cnt = np.bincount(core * ntiles + tl, minlength=NCORES * ntiles).reshape(
        NCORES, ntiles
    )
    nblk = (cnt.max(axis=0) + 127) // 128
    NB = int(nblk.sum())
    tile_slot0 = np.concatenate([[0], np.cumsum(nblk)]) * 128

    out = []
    for c in range(NCORES):
        sel = np.nonzero(core == c)[0]
        tl_c = tl[sel]
        order = np.argsort(tl_c, kind="stable")
        sel_o = sel[order]
        tl_s = tl_c[order]
        start_of = np.searchsorted(tl_s, np.arange(ntiles))
        within = np.arange(len(sel_o)) - start_of[tl_s]
        slot = tile_slot0[tl_s] + within

        src_slots = np.full(NB * 128, zero_row, np.int64)
        src_slots[slot] = src[sel_o]
        lane_slots = np.full(NB * 128, 255, np.int32)
        lane_slots[slot] = lane[sel_o]
        out.append((src_slots, lane_slots))
    return nblk, NB, out


def _prep_dst_type(h, n_per_core, ntiles):
    hdT, hrow = [], []
    for c in range(NCORES):
        rows = h[c * n_per_core : (c + 1) * n_per_core]
        pad = np.zeros((ntiles * 128, D), np.float32)
        pad[: rows.shape[0]] = rows
        t = pad.reshape(ntiles, 128, D)
        hdT.append(np.ascontiguousarray(t.transpose(0, 2, 1)).astype(BF16))
        hrow.append(np.ascontiguousarray(t))
    return hdT, hrow


def _fold_weights(Wk, Wv, Wq, Wa, rel_att, rel_msg, rel_pri, skip):
    sqrt_dk = math.sqrt(DK)
    rel_ts = [0, 1, 0]  # src type: cites: paper, writes: author, rev: paper
    watt, wmsg = [], []
    for e in range(3):
        ts = rel_ts[e]
        ratt = rel_att[e] * (rel_pri[e][:, None, None] / sqrt_dk)
        wa = np.einsum("hiI,hij->Ihj", Wk[ts].reshape(H, DK, D), ratt).reshape(D, D)
        wm = np.einsum("hiI,hij->Ihj", Wv[ts].reshape(H, DK, D), rel_msg[e]).reshape(
            D, D
        )
        watt.append(np.ascontiguousarray(wa).astype(BF16))
        wmsg.append(np.ascontiguousarray(wm).astype(BF16))
    wq = [np.ascontiguousarray(Wq[t].T).astype(BF16) for t in range(2)]
    alpha = 1.0 / (1.0 + np.exp(-skip.astype(np.float64)))
    waT = [
        np.ascontiguousarray(Wa[0].T * alpha[0] * 0.5).astype(BF16),
        np.ascontiguousarray(Wa[1].T * alpha[1]).astype(BF16),
    ]
    return watt, wmsg, wq, waT, alpha


def _build_schedule(nblk_c, nblk_w, nblk_r):
    """Flat block schedule. Returns runs list and per-relation block->flat
    column mapping pieces."""
    runs = []  # (rel, ttype, tile, nb, flat_off, rel_off)
    flat = 0
    for t in range(PT):
        for rel, nblk in ((0, nblk_c), (1, nblk_w)):
            nb = int(nblk[t])
            rel_off = int(nblk[:t].sum())
            if nb:
                runs.append((rel, 0, t, nb, flat, rel_off))
                flat += nb
    for t in range(AT):
        nb = int(nblk_r[t])
        rel_off = int(nblk_r[:t].sum())
        if nb:
            runs.append((2, 1, t, nb, flat, rel_off))
            flat += nb
    return runs, flat


def kernel(**inputs):
    from concourse import bacc, bass, mybir, tile
    from concourse.bass_utils import run_bass_kernel_spmd

    inp = {k: np.asarray(v) for k, v in inputs.items()}
    h_paper = inp["h_paper"].astype(np.float32)
    h_author = inp["h_author"].astype(np.float32)
    for bname in ("bk", "bq", "bv", "ba"):
        assert not np.any(inp[bname]), f"nonzero bias {bname} unsupported"

    watt, wmsg, wq, waT, alpha = _fold_weights(
        inp["Wk"].astype(np.float32), inp["Wv"].astype(np.float32),
        inp["Wq"].astype(np.float32), inp["Wa"].astype(np.float32),
        inp["rel_att"].astype(np.float32), inp["rel_msg"].astype(np.float32),
        inp["rel_pri"].astype(np.float32), inp["skip"].astype(np.float32),
    )

    hp_ext = np.concatenate([h_paper, np.zeros((1, D), np.float32)], 0)
    ha_ext = np.concatenate([h_author, np.zeros((1, D), np.float32)], 0)

    nblk_c, NBC, slots_c = _edge_slots(
        inp["cites_src"].astype(np.int64), inp["cites_dst"].astype(np.int64),
        PPC, PT, NPAP)
    nblk_w, NBW, slots_w = _edge_slots(
        inp["writes_src"].astype(np.int64), inp["writes_dst"].astype(np.int64),
        PPC, PT, NAUT)
    nblk_r, NBR, slots_r = _edge_slots(
        inp["rev_src"].astype(np.int64), inp["rev_dst"].astype(np.int64),
        APC, AT, NPAP)

    runs, NBF = _build_schedule(nblk_c, nblk_w, nblk_r)

    hdT_p, hrow_p = _prep_dst_type(h_paper, PPC, PT)
    hdT_a, hrow_a = _prep_dst_type(h_author, APC, AT)

    # -------- per-core flat streams in schedule order --------
    lane128 = np.arange(128, dtype=np.int32)
    hs_cores, at_cores, Aa_cores = [], [], []
    for c in range(NCORES):
        rel_data = []
        for (h_ext, slots) in ((hp_ext, slots_c), (ha_ext, slots_w),
                               (hp_ext, slots_r)):
            src_slots, lane_slots = slots[c]
            hsT = np.ascontiguousarray(h_ext[src_slots].T).astype(BF16)
            at = (lane128[:, None] == lane_slots[None, :]).astype(BF16)
            nb = len(lane_slots) // 128
            Ab = (lane_slots.reshape(nb, 128)[:, :, None] == lane128).astype(BF16)
            Aa = np.ascontiguousarray(
                Ab.transpose(1, 0, 2).reshape(128, nb * 128))
            rel_data.append((hsT, at, Aa))
        hs_parts, at_parts, Aa_parts = [], [], []
        for (rel, _tt, _t, nb, _f, rel_off) in runs:
            sl = slice(rel_off * 128, (rel_off + nb) * 128)
            hs_parts.append(rel_data[rel][0][:, sl])
            at_parts.append(rel_data[rel][1][:, sl])
            Aa_parts.append(rel_data[rel][2][:, sl])
        hs_cores.append(np.ascontiguousarray(np.concatenate(hs_parts, 1)))
        at_cores.append(np.ascontiguousarray(np.concatenate(at_parts, 1)))
        Aa_cores.append(np.ascontiguousarray(np.concatenate(Aa_parts, 1)))

    # -------- groups (cut at run & chunk boundaries, size <= G) --------
    # block flat idx -> (run idx, j within run)
    groups = []  # (flat_start, n, rel, ttype, tile, run_first, run_last)
    for (rel, tt, t, nb, f0, _ro) in runs:
        i = 0
        while i < nb:
            fs = f0 + i
            n = min(G, nb - i, ((fs // CHUNK) + 1) * CHUNK - fs)
            groups.append(
                (fs, n, rel, tt, t, i == 0, i + n == nb))
            i += n
    NG = len(groups)
    NSB = (NG + NSLOT - 1) // NSLOT

    # -------- build SPMD program --------
    nc = bacc.Bacc("TRN2", target_bir_lowering=False, debug=False,
                   num_devices=NCORES)
    dt = mybir.dt

    d_hs = nc.dram_tensor("hs_flat", [128, NBF * 128], dt.bfloat16,
                          kind="ExternalInput")
    d_at = nc.dram_tensor("at_flat", [128, NBF * 128], dt.bfloat16,
                          kind="ExternalInput")
    d_Aa = nc.dram_tensor("Aa_flat", [128, NBF * 128], dt.bfloat16,
                          kind="ExternalInput")
    d_hdT = {
        0: nc.dram_tensor("hdT_paper", [PT, 128, 128], dt.bfloat16,
                          kind="ExternalInput"),
        1: nc.dram_tensor("hdT_author", [AT, 128, 128], dt.bfloat16,
                          kind="ExternalInput"),
    }
    d_hrow = {
        0: nc.dram_tensor("hrow_paper", [PT, 128, 128], dt.float32,
                          kind="ExternalInput"),
        1: nc.dram_tensor("hrow_author", [AT, 128, 128], dt.float32,
                          kind="ExternalInput"),
    }
    NOUT = (PT + AT) * 128
    d_out = nc.dram_tensor("out", [NOUT, 128], dt.float32, kind="ExternalOutput")

    d_watt = [nc.inline_tensor(watt[e], name=f"watt{e}") for e in range(3)]
    d_wmsg = [nc.inline_tensor(wmsg[e], name=f"wmsg{e}") for e in range(3)]
    d_wq = [nc.inline_tensor(wq[t], name=f"wq{t}") for t in range(2)]
    d_waT = [nc.inline_tensor(waT[t], name=f"waT{t}") for t in range(2)]

    # Hmask64_s [128f, 64]: col m==4s+head(f) -> 1
    hmask_np = []
    headof = (np.arange(128) >> 5)
    for s in range(NSLOT):
        m = (np.arange(64)[None, :] == (4 * s + headof)[:, None])
        hmask_np.append(m.astype(FP16))
    d_hmask = [nc.inline_tensor(hmask_np[s], name=f"hmask{s}")
               for s in range(NSLOT)]
    # HselZ_s [64, 4]: [k, j] = delta(k == 4s+j)  (esc z-extract)
    hselz_np = []
    for s in range(NSLOT):
        m = np.zeros((64, 4), FP16)
        for j in range(4):
            m[4 * s + j, j] = 1
        hselz_np.append(m)
    d_hselz = [nc.inline_tensor(hselz_np[s], name=f"hselz{s}")
               for s in range(NSLOT)]
    # Hsel4 [4, 128] f32: [h, f] = delta(h == head(f))  (ln broadcast)
    hsel4_np = (np.arange(4)[:, None] == headof[None, :]).astype(np.float32)
    d_hsel4 = nc.inline_tensor(hsel4_np, name="hsel4")

    from contextlib import ExitStack

    with tile.TileContext(nc) as tc, ExitStack() as _es:
        _p = lambda *a, **k: _es.enter_context(tc.tile_pool(*a, **k))
        cpool = _p(name="const", bufs=1)
        hs_pool = _p(name="hs_st", bufs=4)
        at_pool = _p(name="at_st", bufs=3)
        Aa_pool = _p(name="Aa_st", bufs=3)
        esc_pool = _p(name="escT", bufs=2)
        prod_pool = _p(name="prodT", bufs=2)
        msg_pool = _p(name="msg", bufs=2)
        qxs_pool = _p(name="qxTs", bufs=2)
        escE_pool = _p(name="escE", bufs=2)
        q_pool = _p(name="qsb", bufs=12)
        hdt_pool = _p(name="hdt", bufs=3)
        t_pool = _p(name="tiles", bufs=2)
        k_ps = _p(name="kps", bufs=2, space="PSUM")
        q_ps_pool = _p(name="qps", bufs=1, space="PSUM")
        sc_ps = _p(name="scps", bufs=1, space="PSUM")
        bankA_pool = _p(name="bankA", bufs=1, space="PSUM")
        bankAgg_pool = _p(name="bankAgg", bufs=1, space="PSUM")
        bankZ_pool = _p(name="bankZ", bufs=1, space="PSUM")
        bankD_pool = _p(name="bankD", bufs=1, space="PSUM")
        if True:
            # constants
            s_watt, s_wmsg = [], []
            for e in range(3):
                a = cpool.tile([128, 128], dt.bfloat16, name=f"s_watt{e}")
                nc.sync.dma_start(out=a[:], in_=d_watt[e][:])
                s_watt.append(a)
                b = cpool.tile([128, 128], dt.bfloat16, name=f"s_wmsg{e}")
                nc.sync.dma_start(out=b[:], in_=d_wmsg[e][:])
                s_wmsg.append(b)
            s_wq, s_waT = [], []
            for t in range(2):
                a = cpool.tile([128, 128], dt.bfloat16, name=f"s_wq{t}")
                nc.sync.dma_start(out=a[:], in_=d_wq[t][:])
                s_wq.append(a)
                b = cpool.tile([128, 128], dt.bfloat16, name=f"s_waT{t}")
                nc.sync.dma_start(out=b[:], in_=d_waT[t][:])
                s_waT.append(b)
            s_hmask = []
            s_hselz = []
            for s in range(NSLOT):
                a = cpool.tile([128, 64], dt.float16, name=f"s_hmask{s}")
                nc.sync.dma_start(out=a[:], in_=d_hmask[s][:])
                s_hmask.append(a)
                b = cpool.tile([64, 4], dt.float16, name=f"s_hselz{s}")
                nc.sync.dma_start(out=b[:], in_=d_hselz[s][:])
                s_hselz.append(b)
            s_hsel4 = cpool.tile([4, 128], dt.float32, name="s_hsel4")
            nc.sync.dma_start(out=s_hsel4[:], in_=d_hsel4[:])
            s_eps = cpool.tile([128, 1], dt.float32, name="s_eps")
            nc.vector.memset(s_eps[:], 1e-30)

            # fixed PSUM tiles. PSUM note: matmul start=True clears
            # has_written for the WHOLE bank, so every multi-matmul
            # accumulation (scores, aggT, zT, out_ps pair) must never have
            # another start=True matmul land in its bank mid-accumulation.
            scores = sc_ps.tile([64, 512], dt.float32, name="scores")
            bankA = bankA_pool.tile([128, 512], dt.float32, name="bankA")
            bankAgg = bankAgg_pool.tile([128, 512], dt.float32, name="bankAgg")
            bankZ = bankZ_pool.tile([128, 512], dt.float32, name="bankZ")
            bankD = bankD_pool.tile([128, 512], dt.float32, name="bankD")
            # bankA layout: v4 [0:512] (one 128-col region per block)
            nc.vector.memset(scores[:, :], 0.0)

            # stream chunk management
            chunk_tiles = {}

            def get_chunk(which, pool, ci):
                key = (which, ci)
                if key in chunk_tiles:
                    return chunk_tiles[key]
                c0 = ci * CHUNK * 128
                w = min(CHUNK * 128, NBF * 128 - c0)
                tl = pool.tile([128, CHUNK * 128], dt.bfloat16, name=which,
                               tag=which)
                src = {"hs": d_hs, "at": d_at, "Aa": d_Aa}[which]
                nc.sync.dma_start(out=tl[:, :w], in_=src[:, c0 : c0 + w])
                chunk_tiles[key] = tl
                return tl

            def chunk_slice(which, pool, fs, n):
                ci, off = divmod(fs, CHUNK)
                tl = get_chunk(which, pool, ci)
                return tl[:, off * 128 : (off + n) * 128]

            # per-tile state
            q_tiles = {}       # (tt, tile) -> Q sbuf tile
            tile_state = {}    # (tt, tile) -> dict(bankC, rels list)

            def emit_q(tt, t):
                key = (tt, t)
                if key in q_tiles:
                    return q_tiles[key]
                hdt = hdt_pool.tile([128, 128], dt.bfloat16, name="hdt",
                                    tag="hdt")
                nc.sync.dma_start(out=hdt[:], in_=d_hdT[tt][t, :, :])
                nc.tensor.matmul(bankD[:, 0:128], lhsT=hdt[:], rhs=s_wq[tt][:],
                                 start=True, stop=True)
                Q = q_pool.tile([128, 128], dt.bfloat16, name="Q", tag="Q")
                nc.scalar.copy(out=Q[:], in_=bankD[:, 0:128])
                q_tiles[key] = Q
                return Q

            def get_tile_state(tt, t):
                key = (tt, t)
                if key not in tile_state:
                    tile_state[key] = {"rels": []}
                return tile_state[key]

            def finalize_tile(tt, t):
                st = tile_state[(tt, t)]
                rels = st["rels"]
                orow = t * 128 if tt == 0 else (PT + t) * 128
                hrow = t_pool.tile([128, 128], dt.float32, name="hrow",
                                   tag="hrow")
                nc.sync.dma_start(out=hrow[:], in_=d_hrow[tt][t, :, :])
                out_s = t_pool.tile([128, 128], dt.float32, name="out_s",
                                    tag="out_s")
                if rels:
                    nr = len(rels)
                    riof = [0, 1, 0]  # bank region per relation
                    c0 = 128 * riof[rels[0]]
                    lzT = t_pool.tile([4, 128 * nr], dt.float32, name="lzT",
                                      tag="lzT")
                    nc.scalar.activation(
                        out=lzT[:], in_=bankZ[0:4, c0 : c0 + 128 * nr],
                        func=mybir.ActivationFunctionType.Ln,
                        bias=s_eps[0:4, 0:1])
                    T_sbs = []
                    for pi, rel in enumerate(rels):
                        ri = riof[rel]
                        # lz_expT [f, d] into bankD[256:384] (f32 matmul)
                        nc.tensor.matmul(
                            bankD[:, 256:384], lhsT=s_hsel4[:],
                            rhs=lzT[:, 128 * pi : 128 * pi + 128],
                            start=True, stop=True)
                        rz_sb = t_pool.tile([128, 128], dt.bfloat16,
                                            name="rz_sb", tag="rz_sb")
                        nc.scalar.activation(
                            out=rz_sb[:], in_=bankD[:, 256:384],
                            func=mybir.ActivationFunctionType.Exp, scale=-1.0)
                        T_sb = t_pool.tile([128, 128], dt.bfloat16, name="T_sb",
                                           tag="T_sb")
                        nc.vector.tensor_tensor(
                            out=T_sb[:],
                            in0=bankAgg[:, 128 * ri : 128 * ri + 128],
                            in1=rz_sb[:], op=mybir.AluOpType.mult)
                        T_sbs.append(T_sb)
                    # out-MM accumulation pair kept adjacent: no other
                    # start=True matmul may land in bankD between them
                    for pi, T_sb in enumerate(T_sbs):
                        nc.tensor.matmul(bankD[:, 128:256], lhsT=T_sb[:],
                                         rhs=s_waT[tt][:],
                                         start=(pi == 0), stop=(pi == nr - 1))
                    nc.vector.scalar_tensor_tensor(
                        out=out_s[:], in0=hrow[:],
                        scalar=float(1.0 - alpha[tt]), in1=bankD[:, 128:256],
                        op0=mybir.AluOpType.mult, op1=mybir.AluOpType.add)
                else:
                    nc.vector.tensor_scalar(
                        out=out_s[:], in0=hrow[:],
                        scalar1=float(1.0 - alpha[tt]), scalar2=None,
                        op0=mybir.AluOpType.mult)
                nc.sync.dma_start(out=d_out[orow : orow + 128, :], in_=out_s[:])
                del tile_state[(tt, t)]

            # main superblock loop
            for sb in range(NSB):
                g0 = sb * NSLOT
                sb_groups = groups[g0 : g0 + NSLOT]
                ns = len(sb_groups)
                # ---- phase A ----
                for s, (fs, n, rel, tt, t, rfirst, rlast) in enumerate(sb_groups):
                    Q = emit_q(tt, t)
                    ec = n * 128
                    hs4 = chunk_slice("hs", hs_pool, fs, n)
                    at4 = chunk_slice("at", at_pool, fs, n)
                    kT = k_ps.tile([128, 512], dt.float32, name="kT", tag="kT")
                    nc.tensor.matmul(kT[:, :ec], lhsT=s_watt[rel][:], rhs=hs4,
                                     start=True, stop=True)
                    qxT = q_ps_pool.tile([128, 512], dt.float32, name="qxT",
                                         tag="qxT")
                    nc.tensor.matmul(qxT[:, :ec], lhsT=Q[:], rhs=at4,
                                     start=True, stop=True)
                    qxTs = qxs_pool.tile([128, 512], dt.float16,
                                         name="qxTs", tag="qxTs")
                    nc.scalar.copy(out=qxTs[:, :ec], in_=qxT[:, :ec])
                    prodT = prod_pool.tile([128, 512], dt.float16, name="prodT",
                                           tag="prodT")
                    nc.vector.tensor_tensor(out=prodT[:, :ec], in0=kT[:, :ec],
                                            in1=qxTs[:, :ec],
                                            op=mybir.AluOpType.mult)
                    nc.tensor.matmul(scores[:, :ec], lhsT=s_hmask[s][:],
                                     rhs=prodT[:, :ec],
                                     start=(s == 0), stop=(s == ns - 1))
                # ---- exp (always full 64 rows: unused rows hold finite
                # stale scores; keeps escT NaN-free for the K=64 lhsT) ----
                escT = esc_pool.tile([64, 512], dt.float16, name="escT",
                                     tag="escT")
                nc.scalar.activation(out=escT[:, :], in_=scores[:, :],
                                     func=mybir.ActivationFunctionType.Exp)
                # ---- phase B ----
                for s, (fs, n, rel, tt, t, rfirst, rlast) in enumerate(sb_groups):
                    st = get_tile_state(tt, t)
                    ri = 0 if rel in (0, 2) else 1
                    if rel not in st["rels"]:
                        st["rels"].append(rel)
                    ec = n * 128
                    # escE [e, 4n] edge-major esc via per-block extract MMs
                    for j in range(n):
                        nc.tensor.matmul(
                            bankD[:, 384 + 4 * j : 388 + 4 * j],
                            lhsT=escT[:, 128 * j : 128 * j + 128],
                            rhs=s_hselz[s][:], start=True, stop=True)
                        hsb = chunk_slice("hs", hs_pool, fs + j, 1)
                        nc.tensor.matmul(
                            bankA[:, 128 * j : 128 * j + 128], lhsT=hsb,
                            rhs=s_wmsg[rel][:], start=True, stop=True)
                    escE = escE_pool.tile([128, 16], dt.bfloat16, name="escE",
                                          tag="escE")
                    nc.vector.tensor_copy(out=escE[:, 0 : 4 * n],
                                          in_=bankD[:, 384 : 384 + 4 * n])
                    msg4 = msg_pool.tile([128, 512], dt.bfloat16, name="msg4",
                                         tag="msg4")
                    nc.vector.tensor_tensor(
                        out=msg4[:, :ec].rearrange("p (x y) -> p x y", y=32),
                        in0=bankA[:, :ec].rearrange("p (x y) -> p x y", y=32),
                        in1=escE[:, 0 : 4 * n].to_broadcast([128, 4 * n, 32]),
                        op=mybir.AluOpType.mult)
                    for j in range(n):
                        first = rfirst and j == 0
                        last = rlast and j == n - 1
                        Ab = chunk_slice("Aa", Aa_pool, fs + j, 1)
                        nc.tensor.matmul(
                            bankAgg[:, 128 * ri : 128 * ri + 128],
                            lhsT=msg4[:, 128 * j : 128 * j + 128], rhs=Ab,
                            start=first, stop=last)
                        nc.tensor.matmul(
                            bankZ[0:4, 128 * ri : 128 * ri + 128],
                            lhsT=escE[:, 4 * j : 4 * j + 4], rhs=Ab,
                            start=first, stop=last)
                    if rlast:
                        # finalize when this was the tile's last relation run
                        is_tile_last = (rel == 2) or (tt == 0 and (
                            rel == 1 or (rel == 0 and nblk_w[t] == 0)))
                        if is_tile_last:
                            finalize_tile(tt, t)

            # tiles with no edges at all: pure skip-blend output
            seen = {(tt, t) for (_r, tt, t, _nb, _f, _ro) in runs}
            for tt, nt in ((0, PT), (1, AT)):
                for t in range(nt):
                    if (tt, t) not in seen:
                        get_tile_state(tt, t)
                        finalize_tile(tt, t)

    nc.compile()

    if os.environ.get("HGT_BUILD_ONLY"):
        return np.zeros((NPAP + NAUT, D), np.float32)

    in_maps = []
    for c in range(NCORES):
        in_maps.append({
            "hs_flat": hs_cores[c], "at_flat": at_cores[c],
            "Aa_flat": Aa_cores[c],
            "hdT_paper": hdT_p[c], "hdT_author": hdT_a[c],
            "hrow_paper": hrow_p[c], "hrow_author": hrow_a[c],
        })

    trace = bool(int(os.environ.get("HGT_TRACE", "0")))
    res = run_bass_kernel_spmd(nc, in_maps, list(range(NCORES)), trace=trace)
    LAST_RESULT["exec_time_ns"] = res.exec_time_ns
    LAST_RESULT["res"] = res
    LAST_RESULT["nc"] = nc
    LAST_RESULT["in_maps"] = in_maps

    out = np.empty((NPAP + NAUT, D), np.float32)
    for c in range(NCORES):
        o = np.asarray(res.results[c]["out"], np.float32)
        out[c * PPC : (c + 1) * PPC] = o[:PPC]
        out[NPAP + c * APC : NPAP + (c + 1) * APC] = o[PT * 128 : PT * 128 + APC]
    return out


# revision 13
# speedup vs baseline: 1.7113x; 1.1038x over previous
"""HGT layer (heterogeneous graph transformer) on 8 Trainium2 NeuronCores.

v2: engine-balanced redesign (v1 was DVE-bound at 3.1ms: ~8.5 vector ops
per 128-edge block, each paying ~150cyc fixed overhead).

Strategy (dst-partitioned, per sharding hint):
  - Dst nodes partitioned contiguously across 8 cores. Host groups edges
    by dst tile (128 dsts), pads to uniform per-tile block budgets, and
    pre-gathers per-edge data into three flat streams (cols = edge slots,
    in flat schedule order):
      hsT [128=feat, NBF*128]   source features, transposed, bf16
      at  [128=dlane, NBF*128]  one-hot A^T (dst lane per edge), bf16
      Aa  [128=elane, NBF*128]  one-hot A (per block: A[e, b*128+d]), bf16
  - Device, per 4-block group (512 edges), scores in TRANSPOSED layout
    (features on partitions, edges on free axis) so DVE fixed costs
    amortize 4x:
      kT   = watt.T @ hsT4          (PE, wkv stationary)
      qxT  = Q.T @ at4              (PE, per-tile Q stationary)
      prodT= kT * qxT               (DVE, one op per 4 blocks, fp16 out)
      scores[4s+h, e] += Hmask64_s.T @ prodT   (PE, per-head col sums)
    Per 16 groups (superblock): one ACT exp -> escT fp16.
  - Per block (edge-major message path):
      esc_full = escT_slice.T @ HselI_s   (PE: [e,132] = esc expanded to
                 per-head cols 0:128 + raw esc at cols 128:132)
      v        = hsT_b.T @ wmsg           (PE, into bank cols 132:260;
                 cols 260:264 pre-set to 1.0)
      msg      = v132 * esc_full          (DVE, one op: [e,0:128]=v*esc,
                 [e,128:132]=esc)
      aggT[f,d] += msg[:,0:128].T @ A_b   (PE, transposed scatter-sum)
      zT[h,d]  += msg[:,128:132].T @ A_b  (PE, softmax denominators)
  - Per dst tile: rz = exp(-ln(z+eps)) on ACT (no table switches:
    ln/exp/copy share one ACT table set), rz_expT = Hsel4.T @ rzT (PE
    partition-broadcast), T = aggT*rz_expT (DVE), out += T.T@WaT (PE,
    accumulated over relations), blend with skip (DVE stt), DMA out.
  Weight folds as v1: rel_att/rel_msg into Wk/Wv; pri/sqrt(dk) into
  attention weights; sigmoid(skip) and 0.5 cross-relation mean into Wa.
"""

import math
import os

import numpy as np
import ml_dtypes

BF16 = ml_dtypes.bfloat16
FP16 = np.float16

NPAP, NAUT = 100000, 50000
D, H, DK = 128, 4, 32
NCORES = 8
PPC, APC = NPAP // NCORES, NAUT // NCORES  # 12500, 6250
PT = (PPC + 127) // 128  # 98 paper tiles / core
AT = (APC + 127) // 128  # 49 author tiles / core

G = 4           # blocks per score group
NSLOT = 16      # groups per superblock (scores psum tile rows = 4*NSLOT)
CHUNK = 32      # blocks per DMA chunk

LAST_RESULT = {}


def _edge_slots(src, dst, n_per_core, ntiles, zero_row):
    """Per-core edge slot assignment grouped by dst tile with uniform
    per-tile block budgets (max over cores). Returns nblk[t] and per-core
    (src_slots, lane_slots) flat arrays of length NB*128."""
    core = dst // n_per_core
    dloc = dst - core * n_per_core
    tl = dloc >> 7
    lane = (dloc & 127).astype(np.int32)

    cnt = np.bincount(core * ntiles + tl, minlength=NCORES * ntiles).reshape(
        NCORES, ntiles
    )
    nblk = (cnt.max(axis=0) + 127) // 128
    NB = int(nblk.sum())
    tile_slot0 = np.concatenate([[0], np.cumsum(nblk)]) * 128

    out = []
    for c in range(NCORES):
        sel = np.nonzero(core == c)[0]
        tl_c = tl[sel]
        order = np.argsort(tl_c, kind="stable")
        sel_o = sel[order]
        tl_s = tl_c[order]
        start_of = np.searchsorted(tl_s, np.arange(ntiles))
        within = np.arange(len(sel_o)) - start_of[tl_s]
        slot = tile_slot0[tl_s] + within

        src_slots = np.full(NB * 128, zero_row, np.int64)
        src_slots[slot] = src[sel_o]
        lane_slots = np.full(NB * 128, 255, np.int32)
        lane_slots[slot] = lane[sel_o]
        out.append((src_slots, lane_slots))
    return nblk, NB, out


def _prep_dst_type(h, n_per_core, ntiles):
    hdT, hrow = [], []
    for c in range(NCORES):
        rows = h[c * n_per_core : (c + 1) * n_per_core]
        pad = np.zeros((ntiles * 128, D), np.float32)
        pad[: rows.shape[0]] = rows
        t = pad.reshape(ntiles, 128, D)
        hdT.append(np.ascontiguousarray(t.transpose(0, 2, 1)).astype(BF16))
        hrow.append(np.ascontiguousarray(t))
    return hdT, hrow


def _fold_weights(Wk, Wv, Wq, Wa, rel_att, rel_msg, rel_pri, skip):
    sqrt_dk = math.sqrt(DK)
    rel_ts = [0, 1, 0]  # src type: cites: paper, writes: author, rev: paper
    watt, wmsg = [], []
    for e in range(3):
        ts = rel_ts[e]
        ratt = rel_att[e] * (rel_pri[e][:, None, None] / sqrt_dk)
        wa = np.einsum("hiI,hij->Ihj", Wk[ts].reshape(H, DK, D), ratt).reshape(D, D)
        wm = np.einsum("hiI,hij->Ihj", Wv[ts].reshape(H, DK, D), rel_msg[e]).reshape(
            D, D
        )
        watt.append(np.ascontiguousarray(wa).astype(BF16))
        wmsg.append(np.ascontiguousarray(wm).astype(BF16))
    wq = [np.ascontiguousarray(Wq[t].T).astype(BF16) for t in range(2)]
    alpha = 1.0 / (1.0 + np.exp(-skip.astype(np.float64)))
    waT = [
        np.ascontiguousarray(Wa[0].T * alpha[0] * 0.5).astype(BF16),
        np.ascontiguousarray(Wa[1].T * alpha[1]).astype(BF16),
    ]
    return watt, wmsg, wq, waT, alpha


def _build_schedule(nblk_c, nblk_w, nblk_r):
    """Flat block schedule. Returns runs list and per-relation block->flat
    column mapping pieces."""
    runs = []  # (rel, ttype, tile, nb, flat_off, rel_off)
    flat = 0
    for t in range(PT):
        for rel, nblk in ((0, nblk_c), (1, nblk_w)):
            nb = int(nblk[t])
            rel_off = int(nblk[:t].sum())
            if nb:
                runs.append((rel, 0, t, nb, flat, rel_off))
                flat += nb
    for t in range(AT):
        nb = int(nblk_r[t])
        rel_off = int(nblk_r[:t].sum())
        if nb:
            runs.append((2, 1, t, nb, flat, rel_off))
            flat += nb
    return runs, flat


def kernel(**inputs):
    from concourse import bacc, bass, mybir, tile
    from concourse.bass_utils import run_bass_kernel_spmd

    inp = {k: np.asarray(v) for k, v in inputs.items()}
    h_paper = inp["h_paper"].astype(np.float32)
    h_author = inp["h_author"].astype(np.float32)
    for bname in ("bk", "bq", "bv", "ba"):
        assert not np.any(inp[bname]), f"nonzero bias {bname} unsupported"

    watt, wmsg, wq, waT, alpha = _fold_weights(
        inp["Wk"].astype(np.float32), inp["Wv"].astype(np.float32),
        inp["Wq"].astype(np.float32), inp["Wa"].astype(np.float32),
        inp["rel_att"].astype(np.float32), inp["rel_msg"].astype(np.float32),
        inp["rel_pri"].astype(np.float32), inp["skip"].astype(np.float32),
    )

    hp_ext = np.concatenate([h_paper, np.zeros((1, D), np.float32)], 0)
    ha_ext = np.concatenate([h_author, np.zeros((1, D), np.float32)], 0)

    nblk_c, NBC, slots_c = _edge_slots(
        inp["cites_src"].astype(np.int64), inp["cites_dst"].astype(np.int64),
        PPC, PT, NPAP)
    nblk_w, NBW, slots_w = _edge_slots(
        inp["writes_src"].astype(np.int64), inp["writes_dst"].astype(np.int64),
        PPC, PT, NAUT)
    nblk_r, NBR, slots_r = _edge_slots(
        inp["rev_src"].astype(np.int64), inp["rev_dst"].astype(np.int64),
        APC, AT, NPAP)

    runs, NBF = _build_schedule(nblk_c, nblk_w, nblk_r)

    hdT_p, hrow_p = _prep_dst_type(h_paper, PPC, PT)
    hdT_a, hrow_a = _prep_dst_type(h_author, APC, AT)

    # -------- per-core flat streams in schedule order --------
    lane128 = np.arange(128, dtype=np.int32)
    hs_cores, at_cores, Aa_cores = [], [], []
    for c in range(NCORES):
        rel_data = []
        for (h_ext, slots) in ((hp_ext, slots_c), (ha_ext, slots_w),
                               (hp_ext, slots_r)):
            src_slots, lane_slots = slots[c]
            hsT = np.ascontiguousarray(h_ext[src_slots].T).astype(BF16)
            at = (lane128[:, None] == lane_slots[None, :]).astype(BF16)
            nb = len(lane_slots) // 128
            Ab = (lane_slots.reshape(nb, 128)[:, :, None] == lane128).astype(BF16)
            Aa = np.ascontiguousarray(
                Ab.transpose(1, 0, 2).reshape(128, nb * 128))
            rel_data.append((hsT, at, Aa))
        hs_parts, at_parts, Aa_parts = [], [], []
        for (rel, _tt, _t, nb, _f, rel_off) in runs:
            sl = slice(rel_off * 128, (rel_off + nb) * 128)
            hs_parts.append(rel_data[rel][0][:, sl])
            at_parts.append(rel_data[rel][1][:, sl])
            Aa_parts.append(rel_data[rel][2][:, sl])
        hs_cores.append(np.ascontiguousarray(np.concatenate(hs_parts, 1)))
        at_cores.append(np.ascontiguousarray(np.concatenate(at_parts, 1)))
        Aa_cores.append(np.ascontiguousarray(np.concatenate(Aa_parts, 1)))

    # -------- groups (cut at run & chunk boundaries, size <= G) --------
    # block flat idx -> (run idx, j within run)
    groups = []  # (flat_start, n, rel, ttype, tile, run_first, run_last)
    for (rel, tt, t, nb, f0, _ro) in runs:
        i = 0
        while i < nb:
            fs = f0 + i
            n = min(G, nb - i, ((fs // CHUNK) + 1) * CHUNK - fs)
            groups.append(
                (fs, n, rel, tt, t, i == 0, i + n == nb))
            i += n
    NG = len(groups)
    NSB = (NG + NSLOT - 1) // NSLOT

    # -------- build SPMD program --------
    nc = bacc.Bacc("TRN2", target_bir_lowering=False, debug=False,
                   num_devices=NCORES)
    dt = mybir.dt

    d_hs = nc.dram_tensor("hs_flat", [128, NBF * 128], dt.bfloat16,
                          kind="ExternalInput")
    d_at = nc.dram_tensor("at_flat", [128, NBF * 128], dt.bfloat16,
                          kind="ExternalInput")
    d_Aa = nc.dram_tensor("Aa_flat", [128, NBF * 128], dt.bfloat16,
                          kind="ExternalInput")
    d_hdT = {
        0: nc.dram_tensor("hdT_paper", [PT, 128, 128], dt.bfloat16,
                          kind="ExternalInput"),
        1: nc.dram_tensor("hdT_author", [AT, 128, 128], dt.bfloat16,
                          kind="ExternalInput"),
    }
    d_hrow = {
        0: nc.dram_tensor("hrow_paper", [PT, 128, 128], dt.float32,
                          kind="ExternalInput"),
        1: nc.dram_tensor("hrow_author", [AT, 128, 128], dt.float32,
                          kind="ExternalInput"),
    }
    NOUT = (PT + AT) * 128
    d_out = nc.dram_tensor("out", [NOUT, 128], dt.float32, kind="ExternalOutput")

    d_watt = [nc.inline_tensor(watt[e], name=f"watt{e}") for e in range(3)]
    d_wmsg = [nc.inline_tensor(wmsg[e], name=f"wmsg{e}") for e in range(3)]
    d_wq = [nc.inline_tensor(wq[t], name=f"wq{t}") for t in range(2)]
    d_waT = [nc.inline_tensor(waT[t], name=f"waT{t}") for t in range(2)]

    # Hmask64_s [128f, 64]: col m==4s+head(f) -> 1
    hmask_np = []
    headof = (np.arange(128) >> 5)
    for s in range(NSLOT):
        m = (np.arange(64)[None, :] == (4 * s + headof)[:, None])
        hmask_np.append(m.astype(FP16))
    d_hmask = [nc.inline_tensor(hmask_np[s], name=f"hmask{s}")
               for s in range(NSLOT)]
    # HselZ_s [64, 4]: [k, j] = delta(k == 4s+j)  (esc z-extract)
    hselz_np = []
    for s in range(NSLOT):
        m = np.zeros((64, 4), FP16)
        for j in range(4):
            m[4 * s + j, j] = 1
        hselz_np.append(m)
    d_hselz = [nc.inline_tensor(hselz_np[s], name=f"hselz{s}")
               for s in range(NSLOT)]
    # Hsel4e [5, 128] f32: rows 0-3 delta(h == head(f)), row 4 = eps
    # (z_expT = Hsel4e.T @ [zT; ones] = z[head(f), d] + eps)
    hsel4_np = np.concatenate([
        (np.arange(4)[:, None] == headof[None, :]).astype(np.float32),
        np.full((1, 128), 1e-30, np.float32)], 0)
    d_hsel4 = nc.inline_tensor(hsel4_np, name="hsel4e")

    from contextlib import ExitStack

    with tile.TileContext(nc) as tc, ExitStack() as _es:
        _p = lambda *a, **k: _es.enter_context(tc.tile_pool(*a, **k))
        cpool = _p(name="const", bufs=1)
        hs_pool = _p(name="hs_st", bufs=4)
        at_pool = _p(name="at_st", bufs=3)
        Aa_pool = _p(name="Aa_st", bufs=3)
        esc_pool = _p(name="escT", bufs=2)
        prod_pool = _p(name="prodT", bufs=2)
        msg_pool = _p(name="msg", bufs=2)
        qxs_pool = _p(name="qxTs", bufs=2)
        escE_pool = _p(name="escE", bufs=2)
        q_pool = _p(name="qsb", bufs=12)
        hdt_pool = _p(name="hdt", bufs=3)
        t_pool = _p(name="tiles", bufs=2)
        k_ps = _p(name="kps", bufs=2, space="PSUM")
        q_ps_pool = _p(name="qps", bufs=1, space="PSUM")
        sc_ps = _p(name="scps", bufs=1, space="PSUM")
        bankA_pool = _p(name="bankA", bufs=1, space="PSUM")
        bankAgg_pool = _p(name="bankAgg", bufs=1, space="PSUM")
        bankZ_pool = _p(name="bankZ", bufs=1, space="PSUM")
        bankD_pool = _p(name="bankD", bufs=1, space="PSUM")
        if True:
            # constants
            s_watt, s_wmsg = [], []
            for e in range(3):
                a = cpool.tile([128, 128], dt.bfloat16, name=f"s_watt{e}")
                nc.sync.dma_start(out=a[:], in_=d_watt[e][:])
                s_watt.append(a)
                b = cpool.tile([128, 128], dt.bfloat16, name=f"s_wmsg{e}")
                nc.sync.dma_start(out=b[:], in_=d_wmsg[e][:])
                s_wmsg.append(b)
            s_wq, s_waT = [], []
            for t in range(2):
                a = cpool.tile([128, 128], dt.bfloat16, name=f"s_wq{t}")
                nc.sync.dma_start(out=a[:], in_=d_wq[t][:])
                s_wq.append(a)
                b = cpool.tile([128, 128], dt.bfloat16, name=f"s_waT{t}")
                nc.sync.dma_start(out=b[:], in_=d_waT[t][:])
                s_waT.append(b)
            _hmask_c, _hselz_c = {}, {}

            def s_hmask(s):
                if s not in _hmask_c:
                    a = cpool.tile([128, 64], dt.float16, name=f"s_hmask{s}")
                    nc.sync.dma_start(out=a[:], in_=d_hmask[s][:])
                    _hmask_c[s] = a
                return _hmask_c[s]

            def s_hselz(s):
                if s not in _hselz_c:
                    b = cpool.tile([64, 4], dt.float16, name=f"s_hselz{s}")
                    nc.sync.dma_start(out=b[:], in_=d_hselz[s][:])
                    _hselz_c[s] = b
                return _hselz_c[s]

            s_hsel4 = cpool.tile([5, 128], dt.float32, name="s_hsel4")
            nc.sync.dma_start(out=s_hsel4[:], in_=d_hsel4[:])

            # fixed PSUM tiles. PSUM note: matmul start=True clears
            # has_written for the WHOLE bank, so every multi-matmul
            # accumulation (scores, aggT, zT, out_ps pair) must never have
            # another start=True matmul land in its bank mid-accumulation.
            scores = sc_ps.tile([64, 512], dt.float32, name="scores")
            bankA = bankA_pool.tile([128, 512], dt.float32, name="bankA")
            bankAgg = bankAgg_pool.tile([128, 512], dt.float32, name="bankAgg")
            bankZ = bankZ_pool.tile([128, 512], dt.float32, name="bankZ")
            bankD = bankD_pool.tile([128, 512], dt.float32, name="bankD")
            # bankA layout: v4 [0:512] (one 128-col region per block)
            nc.vector.memset(scores[:, :], 0.0)
            nc.vector.memset(bankZ[0:8, 0:512], 1.0)

            # stream chunk management
            chunk_tiles = {}

            def get_chunk(which, pool, ci):
                key = (which, ci)
                if key in chunk_tiles:
                    return chunk_tiles[key]
                c0 = ci * CHUNK * 128
                w = min(CHUNK * 128, NBF * 128 - c0)
                tl = pool.tile([128, CHUNK * 128], dt.bfloat16, name=which,
                               tag=which)
                src = {"hs": d_hs, "at": d_at, "Aa": d_Aa}[which]
                nc.sync.dma_start(out=tl[:, :w], in_=src[:, c0 : c0 + w])
                chunk_tiles[key] = tl
                return tl

            def chunk_slice(which, pool, fs, n):
                ci, off = divmod(fs, CHUNK)
                tl = get_chunk(which, pool, ci)
                return tl[:, off * 128 : (off + n) * 128]

            # per-tile state
            q_tiles = {}       # (tt, tile) -> Q sbuf tile
            tile_state = {}    # (tt, tile) -> dict(bankC, rels list)

            def emit_q(tt, t):
                key = (tt, t)
                if key in q_tiles:
                    return q_tiles[key]
                hdt = hdt_pool.tile([128, 128], dt.bfloat16, name="hdt",
                                    tag="hdt")
                nc.sync.dma_start(out=hdt[:], in_=d_hdT[tt][t, :, :])
                nc.tensor.matmul(bankD[:, 0:128], lhsT=hdt[:], rhs=s_wq[tt][:],
                                 start=True, stop=True)
                Q = q_pool.tile([128, 128], dt.bfloat16, name="Q", tag="Q")
                nc.scalar.copy(out=Q[:], in_=bankD[:, 0:128])
                q_tiles[key] = Q
                return Q

            tile_seq = [0]

            def get_tile_state(tt, t):
                key = (tt, t)
                if key not in tile_state:
                    tile_state[key] = {"rels": [], "half": 256 * (tile_seq[0] & 1)}
                    tile_seq[0] += 1
                return tile_state[key]

            def finalize_tile(tt, t):
                st = tile_state[(tt, t)]
                rels = st["rels"]
                orow = t * 128 if tt == 0 else (PT + t) * 128
                hrow = t_pool.tile([128, 128], dt.float32, name="hrow",
                                   tag="hrow")
                nc.sync.dma_start(out=hrow[:], in_=d_hrow[tt][t, :, :])
                out_s = t_pool.tile([128, 128], dt.float32, name="out_s",
                                    tag="out_s")
                if rels:
                    nr = len(rels)
                    hf = st["half"]
                    riof = [0, 1, 0]  # bank region per relation
                    c0 = hf + 128 * riof[rels[0]]
                    # zT rows 0-3 + the persistent ones row 4 -> SBUF
                    zT_sb = t_pool.tile([5, 256], dt.float32, name="zT_sb",
                                        tag="zT_sb")
                    nc.scalar.copy(out=zT_sb[:, 0 : 128 * nr],
                                   in_=bankZ[0:5, c0 : c0 + 128 * nr])
                    T_sbs = []
                    for pi, rel in enumerate(rels):
                        ri = riof[rel]
                        # z_expT + eps [f, d] into bankD[256:384] (f32 matmul)
                        nc.tensor.matmul(
                            bankD[:, 256:384], lhsT=s_hsel4[:],
                            rhs=zT_sb[0:5, 128 * pi : 128 * pi + 128],
                            start=True, stop=True)
                        rz_sb = t_pool.tile([128, 128], dt.float32,
                                            name="rz_sb", tag="rz_sb")
                        nc.vector.reciprocal(out=rz_sb[:],
                                             in_=bankD[:, 256:384])
                        T_sb = t_pool.tile([128, 128], dt.bfloat16, name="T_sb",
                                           tag="T_sb")
                        nc.vector.tensor_tensor(
                            out=T_sb[:],
                            in0=bankAgg[:, hf + 128 * ri : hf + 128 * ri + 128],
                            in1=rz_sb[:], op=mybir.AluOpType.mult)
                        T_sbs.append(T_sb)
                    # out-MM accumulation pair kept adjacent: no other
                    # start=True matmul may land in bankD between them
                    for pi, T_sb in enumerate(T_sbs):
                        nc.tensor.matmul(bankD[:, 128:256], lhsT=T_sb[:],
                                         rhs=s_waT[tt][:],
                                         start=(pi == 0), stop=(pi == nr - 1))
                    nc.vector.scalar_tensor_tensor(
                        out=out_s[:], in0=hrow[:],
                        scalar=float(1.0 - alpha[tt]), in1=bankD[:, 128:256],
                        op0=mybir.AluOpType.mult, op1=mybir.AluOpType.add)
                else:
                    nc.vector.tensor_scalar(
                        out=out_s[:], in0=hrow[:],
                        scalar1=float(1.0 - alpha[tt]), scalar2=None,
                        op0=mybir.AluOpType.mult)
                nc.sync.dma_start(out=d_out[orow : orow + 128, :], in_=out_s[:])
                del tile_state[(tt, t)]

            # main superblock loop
            for sb in range(NSB):
                g0 = sb * NSLOT
                sb_groups = groups[g0 : g0 + NSLOT]
                ns = len(sb_groups)
                # ---- phase A ----
                for s, (fs, n, rel, tt, t, rfirst, rlast) in enumerate(sb_groups):
                    Q = emit_q(tt, t)
                    ec = n * 128
                    hs4 = chunk_slice("hs", hs_pool, fs, n)
                    at4 = chunk_slice("at", at_pool, fs, n)
                    kT = k_ps.tile([128, 512], dt.float32, name="kT", tag="kT")
                    nc.tensor.matmul(kT[:, :ec], lhsT=s_watt[rel][:], rhs=hs4,
                                     start=True, stop=True)
                    qxT = q_ps_pool.tile([128, 512], dt.float32, name="qxT",
                                         tag="qxT")
                    nc.tensor.matmul(qxT[:, :ec], lhsT=Q[:], rhs=at4,
                                     start=True, stop=True)
                    qxTs = qxs_pool.tile([128, 512], dt.float16,
                                         name="qxTs", tag="qxTs")
                    nc.scalar.copy(out=qxTs[:, :ec], in_=qxT[:, :ec])
                    prodT = prod_pool.tile([128, 512], dt.float16, name="prodT",
                                           tag="prodT")
                    nc.vector.tensor_tensor(out=prodT[:, :ec], in0=kT[:, :ec],
                                            in1=qxTs[:, :ec],
                                            op=mybir.AluOpType.mult)
                    nc.tensor.matmul(scores[:, :ec], lhsT=s_hmask(s)[:],
                                     rhs=prodT[:, :ec],
                                     start=(s == 0), stop=(s == ns - 1))
                # ---- exp (always full 64 rows: unused rows hold finite
                # stale scores; keeps escT NaN-free for the K=64 lhsT) ----
                escT = esc_pool.tile([64, 512], dt.float16, name="escT",
                                     tag="escT")
                nc.scalar.activation(out=escT[:, :], in_=scores[:, :],
                                     func=mybir.ActivationFunctionType.Exp)
                # ---- phase B ----
                for s, (fs, n, rel, tt, t, rfirst, rlast) in enumerate(sb_groups):
                    st = get_tile_state(tt, t)
                    ri = 0 if rel in (0, 2) else 1
                    if rel not in st["rels"]:
                        st["rels"].append(rel)
                    ec = n * 128
                    # escE [e, 4n] edge-major esc via per-block extract MMs
                    for j in range(n):
                        nc.tensor.matmul(
                            bankD[:, 384 + 4 * j : 388 + 4 * j],
                            lhsT=escT[:, 128 * j : 128 * j + 128],
                            rhs=s_hselz(s)[:], start=True, stop=True)
                        hsb = chunk_slice("hs", hs_pool, fs + j, 1)
                        nc.tensor.matmul(
                            bankA[:, 128 * j : 128 * j + 128], lhsT=hsb,
                            rhs=s_wmsg[rel][:], start=True, stop=True)
                    escE = escE_pool.tile([128, 16], dt.bfloat16, name="escE",
                                          tag="escE")
                    nc.vector.tensor_copy(out=escE[:, 0 : 4 * n],
                                          in_=bankD[:, 384 : 384 + 4 * n])
                    msg4 = msg_pool.tile([128, 512], dt.bfloat16, name="msg4",
                                         tag="msg4")
                    nc.vector.tensor_tensor(
                        out=msg4[:, :ec].rearrange("p (x y) -> p x y", y=32),
                        in0=bankA[:, :ec].rearrange("p (x y) -> p x y", y=32),
                        in1=escE[:, 0 : 4 * n].to_broadcast([128, 4 * n, 32]),
                        op=mybir.AluOpType.mult)
                    for j in range(n):
                        first = rfirst and j == 0
                        last = rlast and j == n - 1
                        Ab = chunk_slice("Aa", Aa_pool, fs + j, 1)
                        hf = st["half"]
                        nc.tensor.matmul(
                            bankAgg[:, hf + 128 * ri : hf + 128 * ri + 128],
                            lhsT=msg4[:, 128 * j : 128 * j + 128], rhs=Ab,
                            start=first, stop=last)
                        nc.tensor.matmul(
                            bankZ[0:4, hf + 128 * ri : hf + 128 * ri + 128],
                            lhsT=escE[:, 4 * j : 4 * j + 4], rhs=Ab,
                            start=first, stop=last)
                    if rlast:
                        # finalize when this was the tile's last relation run
                        is_tile_last = (rel == 2) or (tt == 0 and (
                            rel == 1 or (rel == 0 and nblk_w[t] == 0)))
                        if is_tile_last:
                            finalize_tile(tt, t)

            # tiles with no edges at all: pure skip-blend output
            seen = {(tt, t) for (_r, tt, t, _nb, _f, _ro) in runs}
            for tt, nt in ((0, PT), (1, AT)):
                for t in range(nt):
                    if (tt, t) not in seen:
                        get_tile_state(tt, t)
                        finalize_tile(tt, t)

    nc.compile()

    if os.environ.get("HGT_BUILD_ONLY"):
        return np.zeros((NPAP + NAUT, D), np.float32)

    in_maps = []
    for c in range(NCORES):
        in_maps.append({
            "hs_flat": hs_cores[c], "at_flat": at_cores[c],
            "Aa_flat": Aa_cores[c],
            "hdT_paper": hdT_p[c], "hdT_author": hdT_a[c],
            "hrow_paper": hrow_p[c], "hrow_author": hrow_a[c],
        })

    trace = bool(int(os.environ.get("HGT_TRACE", "0")))
    res = run_bass_kernel_spmd(nc, in_maps, list(range(NCORES)), trace=trace)
    LAST_RESULT["exec_time_ns"] = res.exec_time_ns
    LAST_RESULT["res"] = res
    LAST_RESULT["nc"] = nc
    LAST_RESULT["in_maps"] = in_maps

    out = np.empty((NPAP + NAUT, D), np.float32)
    for c in range(NCORES):
        o = np.asarray(res.results[c]["out"], np.float32)
        out[c * PPC : (c + 1) * PPC] = o[:PPC]
        out[NPAP + c * APC : NPAP + (c + 1) * APC] = o[PT * 128 : PT * 128 + APC]
    return out
